# revision 3
# baseline (speedup 1.0000x reference)
"""CoedgeConvLayer Trainium2 kernel.

y = relu(x @ W_self + x[next] @ W_next + x[prev] @ W_prev + x[mate] @ W_mate + b)

Sharding: coedge rows data-parallel across 8 NeuronCores; the bf16 feature
table is replicated per core so neighbor gathers are purely local.  Per
128-row subtile:
  - 3 neighbor-stream tiles come from [P,1]-offset SWDGE indirect gathers
    (one row per partition; the only gather form this DGE supports), bf16
    to halve gather bytes.
  - The self stream needs no gather: each core gets its own row-slice of
    the table pre-transposed host-side (featsT, [2*128, 25088]) and loads
    it with plain strided HWDGE DMA, already in lhsT layout.
  - The 6 neighbor [128,128] chunks are transposed on the PE via regular
    identity matmuls into one f32 PSUM tile (transpose-mode with bf16 PSUM
    output is unreliable on HW) and moved/cast to bf16 SBUF by one DVE copy.
  - Accumulation: a K=1 ones-x-bias matmul initializes PSUM, then 2 self +
    6 neighbor bf16 matmuls accumulate; ReLU + bf16 cast ride the
    PSUM->SBUF activation; output is stored bf16 and widened on host.
The kernel is SWDGE-bound: 3 x 196 = 588 indirect gathers x ~1.06us fixed
descriptor-generation cost on the Pool engine ~= the 624us runtime; all
other engines (PE ~35%, DMA ~30%, DVE/ACT <15%) hide underneath.
"""

import os

import numpy as np
import ml_dtypes

import concourse.bass as bass
from concourse import bacc
import concourse.mybir as mybir
import concourse.tile as tile
from concourse import bass_utils

N = 200000
D = 256
NCORES = 8
ROWS_PER_CORE = N // NCORES          # 25000
P = 128
SUBTILES = (ROWS_PER_CORE + P - 1) // P   # 196
PAD_ROWS = SUBTILES * P              # 25088
G = 7                                # subtiles per block
NBLOCKS = SUBTILES // G              # 28
KCHUNKS = 2
NNBR = 3                             # neighbor streams: next, prev, mate
NPAD = ((NCORES - 1) * ROWS_PER_CORE + PAD_ROWS + P - 1) // P * P  # 200192

GATHER_FORM = os.environ.get("KERNEL_GATHER", "p1")
PT_F32 = os.environ.get("KERNEL_PTF32", "0") == "1"
BIAS_LAST = os.environ.get("KERNEL_BIASLAST", "0") == "1"
SELF_GATHER = os.environ.get("KERNEL_SELFGATHER", "0") == "1"
OUT_F32 = os.environ.get("KERNEL_OUTF32", "0") == "1"
PT_DT = None  # set in _build_nc
GBUFS = int(os.environ.get("KERNEL_GBUFS", "9"))
XTBUFS = int(os.environ.get("KERNEL_XTBUFS", "3"))

BF = mybir.dt.bfloat16
NP_BF = ml_dtypes.bfloat16


VER = int(os.environ.get("KERNEL_VER", "3"))


def _build_nc(repeat=1):
    nc = bacc.Bacc("TRN2", debug=False, enable_partition_id=False)
    # Executable caches key on tensor shapes; a version-sized dummy input
    # guarantees a fresh compile per kernel revision.
    vtag = nc.dram_tensor("vtag", [1, VER], mybir.dt.int32,
                          kind="ExternalInput")
    feats = nc.dram_tensor("features", [NPAD, D], BF, kind="ExternalInput")
    featsT = nc.dram_tensor("featsT", [KCHUNKS * P, PAD_ROWS], BF,
                            kind="ExternalInput")
    w = nc.dram_tensor("w", [4 * D, D], BF, kind="ExternalInput")
    bias = nc.dram_tensor("bias", [1, D], BF, kind="ExternalInput")
    # neighbor idx: [P, NBLOCKS * NNBR * G]
    idx = nc.dram_tensor("idx", [P, NBLOCKS * NNBR * G], mybir.dt.int32,
                         kind="ExternalInput")
    out_dt = mybir.dt.float32 if OUT_F32 else BF
    pt_dt = mybir.dt.float32 if PT_F32 else BF
    out = nc.dram_tensor("out", [PAD_ROWS, D], out_dt, kind="ExternalOutput")

    feats_ap = feats.ap()
    out_ap = out.ap()
    SG = NNBR * G

    with tile.TileContext(nc) as tc:
        with (
            tc.tile_pool(name="const", bufs=1) as cpool,
            tc.tile_pool(name="selfp", bufs=2) as spool,
            tc.tile_pool(name="gather", bufs=GBUFS) as gpool,
            tc.tile_pool(name="xt", bufs=XTBUFS) as xtpool,
            tc.tile_pool(name="outp", bufs=2) as opool,
            tc.tile_pool(name="pt", bufs=2, space="PSUM") as ptpool,
            tc.tile_pool(name="pacc", bufs=3, space="PSUM") as paccpool,
        ):
            # Resident constants.
            w_sb = cpool.tile([P, 4 * KCHUNKS, D], BF)
            nc.sync.dma_start(
                out=w_sb[:], in_=w.ap().rearrange("(c p) n -> p c n", p=P))
            bias_sb = cpool.tile([1, D], BF)
            nc.sync.dma_start(out=bias_sb[:], in_=bias.ap())
            idx_sb = cpool.tile([P, NBLOCKS * SG], mybir.dt.int32)
            nc.sync.dma_start(out=idx_sb[:], in_=idx.ap())
            vtag_sb = cpool.tile([1, VER], mybir.dt.int32)
            nc.sync.dma_start(out=vtag_sb[:], in_=vtag.ap())
            from concourse.masks import make_identity
            ident = cpool.tile([P, P], BF)
            make_identity(nc, ident[:])
            ones_sb = cpool.tile([1, P], BF)
            nc.gpsimd.memset(ones_sb[:], 1.0)
            # Priming matmul (folds gpsimd-preamble wait into PE's vector
            # clock; see v1 docstring).
            pt0 = ptpool.tile([P, NNBR * KCHUNKS, P], mybir.dt.float32,
                              tag='pt')
            nc.tensor.matmul(pt0[:, 0, :], lhsT=ident[:], rhs=ident[:])

            for b in range(NBLOCKS * repeat):
                b = b % NBLOCKS
                r0 = b * G * P
                # Self stream, transposed layout: [P, KCHUNKS, G*P].
                selfT = spool.tile([P, KCHUNKS, G * P], BF)
                nc.sync.dma_start(
                    out=selfT[:],
                    in_=featsT.ap()[:, r0:r0 + G * P].rearrange(
                        "(c p) n -> p c n", p=P))
                outsb = opool.tile([P, G, D], BF)
                for g in range(G):
                    # Gather the 3 neighbor-stream subtiles.
                    srcs = []
                    for s in range(NNBR):
                        col = b * SG + s * G + g
                        xgt = gpool.tile([P, D], BF, tag="xg")
                        nc.gpsimd.indirect_dma_start(
                            out=xgt[:],
                            out_offset=None,
                            in_=feats_ap,
                            in_offset=bass.IndirectOffsetOnAxis(
                                ap=idx_sb[:, col:col + 1], axis=0),
                        )
                        srcs.append(xgt[:])
                    # Transpose 6 chunks into one PSUM tile via regular
                    # identity matmuls (bf16 in, f32 PSUM out — the
                    # transpose-mode bf16 PSUM path is unreliable on HW).
                    pt = ptpool.tile([P, NNBR * KCHUNKS, P], mybir.dt.float32,
                                     tag='pt')
                    for s in range(NNBR):
                        for ki in range(KCHUNKS):
                            nc.tensor.matmul(
                                pt[:, s * KCHUNKS + ki, :],
                                lhsT=srcs[s][:, ki * P:(ki + 1) * P],
                                rhs=ident[:])
                    xt = xtpool.tile([P, NNBR * KCHUNKS, P], BF)
                    nc.vector.tensor_copy(out=xt[:], in_=pt[:])
                    # Accumulate: bias (start) + 2 self + 6 neighbor matmuls.
                    pacc = paccpool.tile([P, D], mybir.dt.float32)
                    if not BIAS_LAST:
                        nc.tensor.matmul(
                            pacc[:], lhsT=ones_sb[:1, :], rhs=bias_sb[:1, :],
                            start=True, stop=False)
                    for ki in range(KCHUNKS):
                        nc.tensor.matmul(
                            pacc[:],
                            lhsT=selfT[:, ki, g * P:(g + 1) * P],
                            rhs=w_sb[:, ki, :],
                            start=(BIAS_LAST and ki == 0), stop=False)
                    for j in range(NNBR * KCHUNKS):
                        s, ki = divmod(j, KCHUNKS)
                        nc.tensor.matmul(
                            pacc[:], lhsT=xt[:, j, :],
                            rhs=w_sb[:, (s + 1) * KCHUNKS + ki, :],
                            start=False,
                            stop=(not BIAS_LAST and j == NNBR * KCHUNKS - 1))
                    if BIAS_LAST:
                        nc.tensor.matmul(
                            pacc[:], lhsT=ones_sb[:1, :], rhs=bias_sb[:1, :],
                            start=False, stop=True)
                    nc.scalar.activation(
                        outsb[:, g, :], pacc[:],
                        mybir.ActivationFunctionType.Relu)
                nc.sync.dma_start(
                    out=out_ap[r0:r0 + G * P, :].rearrange(
                        "(g p) n -> p g n", p=P),
                    in_=outsb[:],
                )
    nc.compile()
    return nc


def _prepare_in_maps(features, next_indices, prev_indices, mate_indices,
                     W_self, b_self, W_next, b_next, W_prev, b_prev,
                     W_mate, b_mate):
    feats = np.zeros((NPAD, D), dtype=NP_BF)
    feats[:N] = np.asarray(features, np.float32).astype(NP_BF)          # [D, NPAD] bf16

    w_cat = np.concatenate(
        [np.asarray(W_self, np.float32), np.asarray(W_next, np.float32),
         np.asarray(W_prev, np.float32), np.asarray(W_mate, np.float32)],
        axis=0).astype(NP_BF)
    w_cat = np.ascontiguousarray(w_cat)
    b_tot = (np.asarray(b_self, np.float32) + np.asarray(b_next, np.float32)
             + np.asarray(b_prev, np.float32) + np.asarray(b_mate, np.float32))
    b_tot = np.ascontiguousarray(b_tot.reshape(1, D).astype(NP_BF))

    nbr = [np.asarray(next_indices), np.asarray(prev_indices),
           np.asarray(mate_indices)]

    in_maps = []
    for c in range(NCORES):
        base = c * ROWS_PER_CORE
        # idx layout: [P, NBLOCKS, NNBR, G]; local row r = b*G*P + g*P + p.
        idx_arr = np.zeros((P, NBLOCKS, NNBR, G), dtype=np.int32)
        for s, I in enumerate(nbr):
            loc = np.zeros(PAD_ROWS, dtype=np.int64)
            loc[:ROWS_PER_CORE] = I[base:base + ROWS_PER_CORE]
            idx_arr[:, :, s, :] = (
                loc.reshape(NBLOCKS, G, P).transpose(2, 0, 1))
        idx_flat = np.ascontiguousarray(
            idx_arr.reshape(P, NBLOCKS * NNBR * G))
        featsT_c = np.ascontiguousarray(
            feats[base:base + PAD_ROWS].T)
        in_maps.append({
            "vtag": np.zeros((1, VER), np.int32),
            "features": feats,
            "featsT": featsT_c,
            "w": w_cat,
            "bias": b_tot,
            "idx": idx_flat,
        })
    return in_maps


def _unpad_output(results):
    out = np.concatenate(
        [np.asarray(results[c]["out"][:ROWS_PER_CORE], np.float32)
         for c in range(NCORES)], axis=0)
    return np.ascontiguousarray(out)


def kernel(**inputs) -> np.ndarray:
    in_maps = _prepare_in_maps(**inputs)
    nc = _build_nc()
    res = bass_utils.run_bass_kernel_spmd(
        nc, in_maps, core_ids=list(range(NCORES)))
    return _unpad_output(res.results)
